# revision 13
# baseline (speedup 1.0000x reference)
"""Trainium2 Bass kernel for nn_MultiHeadAttention (B=4, S=1024, E=1024, H=16, D=64).

Returns (out, att_weights), matching the reference.

Sharding: 8 cores = 4 batches x 2 head-groups. Core c handles batch c//2 and
heads [g*8, (g+1)*8) where g = c%2, via column-parallel Wq/Wk/Wv (columns
g*512:(g+1)*512) and row-parallel Wo (rows g*512:(g+1)*512). Host sums the two
partial output projections per batch and adds (bv @ Wo + bo) — the bv term
folds out of the device computation because softmax rows sum to 1.

Per-core pipeline (all fp32):
  P1: PE-transpose q,k,v batch slices to [E, S] layout; project to
      qh^T/kh^T [d, s] (bias added on PSUM->SBUF copy) and vh [s, d].
  P2: per head: logits = qh^T.T @ kh^T (K=64 matmul), exp via ACT with
      fused 1/sqrt(D) scale and accum_out row-sum (softmax max-subtraction is
      skipped: |logits| < ~3 for this model family), normalize with per-row
      reciprocal, DMA att out, PE-transpose att, ctx^T = vh.T-contraction.
      Heads are processed in pairs sharing the 128-partition dim (row/col
      tile packing).
  P3: out_partial = ctx^T.T @ Wo_slice.
"""

import sys

import numpy as np

if "/opt/trn_rl_repo" not in sys.path:
    sys.path.insert(0, "/opt/trn_rl_repo")

P = 128

# Full-problem config
FULL = dict(B=4, S=1024, E=1024, H=16, D=64, GROUPS=2)


def _cfg(B, S, E, H, D, GROUPS):
    DG = (H // GROUPS) * D  # d per core
    return dict(
        B=B, S=S, E=E, H=H, D=D, GROUPS=GROUPS,
        DG=DG,
        HL=H // GROUPS,       # local heads
        EC=E // P,            # E chunks (contraction tiles)
        ST=S // P,            # S tiles
        MT=DG // P,           # d m-tiles (2 heads per m-tile when D=64)
        NW=min(512, S),       # matmul moving width
    )


def build_core_kernel(use_f32r: bool = False, cfg: dict | None = None):
    """Build and compile the per-core Bass program (same program on all cores)."""
    import concourse.tile as tile
    from concourse import bacc, mybir
    from concourse.masks import make_identity

    c = _cfg(**(cfg or FULL))
    S, E, DG = c["S"], c["E"], c["DG"]
    EC, ST, MT, NW = c["EC"], c["ST"], c["MT"], c["NW"]
    HL = c["HL"]
    D = c["D"]
    assert D == 64

    f32 = mybir.dt.float32
    mm_dt = mybir.dt.float32r if use_f32r else f32
    Exp = mybir.ActivationFunctionType.Exp

    nc = bacc.Bacc("TRN2", target_bir_lowering=False, debug=False)

    q_ap = nc.dram_tensor("q", [S, E], f32, kind="ExternalInput").ap()
    k_ap = nc.dram_tensor("k", [S, E], f32, kind="ExternalInput").ap()
    v_ap = nc.dram_tensor("v", [S, E], f32, kind="ExternalInput").ap()
    wq_ap = nc.dram_tensor("wq", [E, DG], f32, kind="ExternalInput").ap()
    wk_ap = nc.dram_tensor("wk", [E, DG], f32, kind="ExternalInput").ap()
    wv_ap = nc.dram_tensor("wv", [E, DG], f32, kind="ExternalInput").ap()
    bq_ap = nc.dram_tensor("bq", [DG], f32, kind="ExternalInput").ap()
    bk_ap = nc.dram_tensor("bk", [DG], f32, kind="ExternalInput").ap()
    wo_ap = nc.dram_tensor("wo", [DG, E], f32, kind="ExternalInput").ap()
    att_ap = nc.dram_tensor("att", [HL, S, S], f32, kind="ExternalOutput").ap()
    outp_ap = nc.dram_tensor("outp", [S, E], f32, kind="ExternalOutput").ap()

    TQ = min(4, EC)   # transposes batched per PSUM tile
    SQ = min(4, ST)

    with tile.TileContext(nc) as tc:
        with (
            tc.tile_pool(name="const", bufs=1) as constp,
            tc.tile_pool(name="persist", bufs=1) as persist,
            tc.tile_pool(name="lp", bufs=8) as lp,
            tc.tile_pool(name="psA", bufs=2, space="PSUM") as psA,
            tc.tile_pool(name="psB", bufs=2, space="PSUM") as psB,
            tc.tile_pool(name="psC", bufs=2, space="PSUM") as psC,
        ):
            ident = constp.tile([P, P], f32)
            make_identity(nc, ident)

            bq_sb = constp.tile([P, MT], f32, tag="bq")
            bk_sb = constp.tile([P, MT], f32, tag="bk")
            nc.gpsimd.dma_start(bq_sb[:], bq_ap.rearrange("(m p) -> p m", p=P))
            nc.gpsimd.dma_start(bk_sb[:], bk_ap.rearrange("(m p) -> p m", p=P))

            qhT = persist.tile([P, MT, S], mm_dt, tag="qhT")
            khT = persist.tile([P, MT, S], mm_dt, tag="khT")
            vh = persist.tile([P, ST, DG], mm_dt, tag="vh")
            ctxT = persist.tile([P, MT, S], mm_dt, tag="ctxT")

            # ---------------- Phase 1: transpose inputs + projections ----------
            with (
                tc.tile_pool(name="xin", bufs=4) as xinp,
                tc.tile_pool(name="xt", bufs=2) as xtp,
                tc.tile_pool(name="w", bufs=2) as wp,
            ):
                for xi, (x_ap, w_ap2, out_sb, bias) in enumerate((
                    (q_ap, wq_ap, qhT, bq_sb),
                    (k_ap, wk_ap, khT, bk_sb),
                    (v_ap, wv_ap, vh, None),
                )):
                    _sid = nc.enter_named_scope(f"p1_{'qkv'[xi]}", False)[0]
                    w_sb = None

                    xT = xtp.tile([P, EC, S], mm_dt, tag="xT")
                    for st in range(ST):
                        xin = xinp.tile([P, E], f32, tag="xin")
                        nc.sync.dma_start(xin[:], x_ap[st * P : (st + 1) * P, :])
                        if st == 0:
                            # issue the weight DMA behind the first input chunk
                            # so it doesn't head-block the transpose pipeline
                            w_sb = wp.tile([P, EC, DG], f32, tag="w")
                            nc.sync.dma_start(
                                w_sb[:], w_ap2.rearrange("(e p) d -> p e d", p=P)
                            )
                            if use_f32r:
                                w_mm = wp.tile([P, EC, DG], mm_dt, tag="wr")
                                nc.vector.tensor_copy(w_mm[:], w_sb[:])
                            else:
                                w_mm = w_sb
                        for eq in range(EC // TQ):
                            pst = psB.tile([P, TQ * P], f32, tag="psB")
                            for t in range(TQ):
                                e = eq * TQ + t
                                nc.tensor.transpose(
                                    pst[:, t * P : (t + 1) * P],
                                    xin[:, e * P : (e + 1) * P],
                                    ident[:],
                                )
                            nc.vector.tensor_copy(
                                xT[:, eq * TQ : (eq + 1) * TQ, st * P : (st + 1) * P],
                                pst[:].rearrange("p (t c) -> p t c", c=P),
                            )

                    if bias is not None:
                        # transposed projection: out [d, s] = w.T @ x.T
                        for m in range(MT):
                            psp = psA.tile([P, S], f32, tag="psA")
                            for sh in range(S // NW):
                                for e in range(EC):
                                    nc.tensor.matmul(
                                        psp[:, sh * NW : (sh + 1) * NW],
                                        w_mm[:, e, m * P : (m + 1) * P],
                                        xT[:, e, sh * NW : (sh + 1) * NW],
                                        start=(e == 0),
                                        stop=(e == EC - 1),
                                    )
                            nc.vector.tensor_scalar_add(
                                out_sb[:, m, :], psp[:], bias[:, m : m + 1]
                            )
                    else:
                        # natural projection: out [s, d] = x @ w
                        for st in range(ST):
                            psp = psA.tile([P, S], f32, tag="psA")
                            for dh in range((DG + NW - 1) // NW):
                                nw = min(NW, DG - dh * NW)
                                for e in range(EC):
                                    nc.tensor.matmul(
                                        psp[:, dh * NW : dh * NW + nw],
                                        xT[:, e, st * P : (st + 1) * P],
                                        w_mm[:, e, dh * NW : dh * NW + nw],
                                        start=(e == 0),
                                        stop=(e == EC - 1),
                                    )
                            nc.vector.tensor_copy(out_sb[:, st, :], psp[:, :DG])
                    nc.leave_named_scope(f"p1_{'qkv'[xi]}", _sid, False)

            # ---------------- Phase 2: attention, head pairs -------------------
            with (
                tc.tile_pool(name="att", bufs=4) as attp,
                tc.tile_pool(name="attT", bufs=3) as attTp,
            ):
                for t in range(MT):
                    _sid2 = nc.enter_named_scope(f"p2_pair{t}", False)[0]
                    attT = [
                        attTp.tile([P, ST, S], mm_dt, tag="attT", name=f"attT_{t}_{hp}")
                        for hp in range(2)
                    ]
                    for i in range(ST):
                        for hp in range(2):
                            h = 2 * t + hp
                            hb = hp * 64
                            psl = psA.tile([P, S], f32, tag="psA")
                            for jh in range(S // NW):
                                nc.tensor.matmul(
                                    psl[:, jh * NW : (jh + 1) * NW],
                                    qhT[hb : hb + 64, t, i * P : (i + 1) * P],
                                    khT[hb : hb + 64, t, jh * NW : (jh + 1) * NW],
                                    start=True,
                                    stop=True,
                                )
                            e_t = attp.tile([P, S], f32, tag="att")
                            l_t = lp.tile([P, 1], f32, tag="l")
                            nc.scalar.activation(
                                e_t[:], psl[:], Exp, scale=0.125, accum_out=l_t[:]
                            )
                            rl_t = lp.tile([P, 1], f32, tag="rl")
                            nc.vector.reciprocal(rl_t[:], l_t[:])
                            a_t = attp.tile([P, S], f32, tag="att")
                            nc.any.tensor_scalar_mul(a_t[:], e_t[:], rl_t[:])
                            nc.sync.dma_start(
                                att_ap[h, i * P : (i + 1) * P, :], a_t[:]
                            )
                            for jq in range(ST // SQ):
                                pst = psB.tile([P, SQ * P], f32, tag="psB")
                                for tt in range(SQ):
                                    jt = jq * SQ + tt
                                    nc.tensor.transpose(
                                        pst[:, tt * P : (tt + 1) * P],
                                        a_t[:, jt * P : (jt + 1) * P],
                                        ident[:],
                                    )
                                nc.vector.tensor_copy(
                                    attT[hp][
                                        :, jq * SQ : (jq + 1) * SQ, i * P : (i + 1) * P
                                    ],
                                    pst[:].rearrange("p (t c) -> p t c", c=P),
                                )
                    # ctx^T for the head pair, col-packed into one PSUM tile
                    for ih in range(S // NW):
                        psc = psC.tile([P, NW], f32, tag="psC")
                        for jt in range(ST):
                            for hp in range(2):
                                h = 2 * t + hp
                                hb = hp * 64
                                nc.tensor.matmul(
                                    psc[hb : hb + 64, :],
                                    vh[:, jt, h * 64 : (h + 1) * 64],
                                    attT[hp][:, jt, ih * NW : (ih + 1) * NW],
                                    start=(jt == 0),
                                    stop=(jt == ST - 1),
                                    tile_position=(0, hb) if hb else None,
                                    skip_group_check=True,
                                )
                        nc.vector.tensor_copy(
                            ctxT[:, t, ih * NW : (ih + 1) * NW], psc[:]
                        )
                    nc.leave_named_scope(f"p2_pair{t}", _sid2, False)

            # ---------------- Phase 3: output projection -----------------------
            with (
                tc.tile_pool(name="w3", bufs=1) as w3p,
                tc.tile_pool(name="outp", bufs=3) as outpp,
            ):
                _sid3 = nc.enter_named_scope("p3_outproj", False)[0]
                wo_sb = w3p.tile([P, MT, E], f32, tag="wo")
                nc.sync.dma_start(wo_sb[:], wo_ap.rearrange("(m p) o -> p m o", p=P))
                if use_f32r:
                    wo_mm = w3p.tile([P, MT, E], mm_dt, tag="wor")
                    nc.vector.tensor_copy(wo_mm[:], wo_sb[:])
                else:
                    wo_mm = wo_sb
                for i in range(ST):
                    pso = psA.tile([P, E], f32, tag="psA")
                    for oh in range(E // NW):
                        for m in range(MT):
                            nc.tensor.matmul(
                                pso[:, oh * NW : (oh + 1) * NW],
                                ctxT[:, m, i * P : (i + 1) * P],
                                wo_mm[:, m, oh * NW : (oh + 1) * NW],
                                start=(m == 0),
                                stop=(m == MT - 1),
                            )
                    o_t = outpp.tile([P, E], f32, tag="out")
                    nc.vector.tensor_copy(o_t[:], pso[:])
                    nc.sync.dma_start(outp_ap[i * P : (i + 1) * P, :], o_t[:])
                nc.leave_named_scope("p3_outproj", _sid3, False)

    nc.compile()
    return nc


_NC_CACHE: dict = {}


def _get_nc(use_f32r: bool = False):
    key = ("full", use_f32r)
    if key not in _NC_CACHE:
        _NC_CACHE[key] = build_core_kernel(use_f32r=use_f32r)
    return _NC_CACHE[key]


def kernel(q, k, v, Wq, bq, Wk, bk, Wv, bv, Wo, bo):
    from concourse.bass_utils import run_bass_kernel_spmd

    B, S, E = FULL["B"], FULL["S"], FULL["E"]
    H, GROUPS = FULL["H"], FULL["GROUPS"]
    HL = H // GROUPS
    DG = HL * FULL["D"]

    q = np.asarray(q, dtype=np.float32)
    k = np.asarray(k, dtype=np.float32)
    v = np.asarray(v, dtype=np.float32)
    Wq = np.asarray(Wq, dtype=np.float32)
    Wk = np.asarray(Wk, dtype=np.float32)
    Wv = np.asarray(Wv, dtype=np.float32)
    Wo = np.asarray(Wo, dtype=np.float32)
    bq = np.asarray(bq, dtype=np.float32)
    bk = np.asarray(bk, dtype=np.float32)
    bv = np.asarray(bv, dtype=np.float32)
    bo = np.asarray(bo, dtype=np.float32)

    nc = _get_nc()

    in_maps = []
    for c in range(8):
        b, g = divmod(c, 2)
        sl = slice(g * DG, (g + 1) * DG)
        in_maps.append(
            {
                "q": np.ascontiguousarray(q[b]),
                "k": np.ascontiguousarray(k[b]),
                "v": np.ascontiguousarray(v[b]),
                "wq": np.ascontiguousarray(Wq[:, sl]),
                "wk": np.ascontiguousarray(Wk[:, sl]),
                "wv": np.ascontiguousarray(Wv[:, sl]),
                "bq": np.ascontiguousarray(bq[sl]),
                "bk": np.ascontiguousarray(bk[sl]),
                "wo": np.ascontiguousarray(Wo[sl, :]),
            }
        )

    res = run_bass_kernel_spmd(nc, in_maps, list(range(8)))

    att = np.empty((B, H, S, S), dtype=np.float32)
    out = np.empty((B, S, E), dtype=np.float32)
    corr = (bv.astype(np.float64) @ Wo.astype(np.float64) + bo).astype(np.float32)
    for c in range(8):
        b, g = divmod(c, 2)
        att[b, g * HL : (g + 1) * HL] = res.results[c]["att"]
    for b in range(B):
        out[b] = res.results[2 * b]["outp"] + res.results[2 * b + 1]["outp"] + corr
    return out, att


# revision 16
# speedup vs baseline: 1.0307x; 1.0307x over previous
"""Trainium2 Bass kernel for nn_MultiHeadAttention (B=4, S=1024, E=1024, H=16, D=64).

Returns (out, att_weights), matching the reference.

Sharding: 8 cores = 4 batches x 2 head-groups. Core c handles batch c//2 and
heads [g*8, (g+1)*8) where g = c%2, via column-parallel Wq/Wk/Wv (columns
g*512:(g+1)*512) and row-parallel Wo (rows g*512:(g+1)*512). Host sums the two
partial output projections per batch and adds (bv @ Wo + bo) — the bv term
folds out of the device computation because softmax rows sum to 1.

Per-core pipeline (all fp32):
  P1: PE-transpose q,k,v batch slices to [E, S] layout; project to
      qh^T/kh^T [d, s] (bias added on PSUM->SBUF copy) and vh [s, d].
  P2: per head: logits = qh^T.T @ kh^T (K=64 matmul), exp via ACT with
      fused 1/sqrt(D) scale and accum_out row-sum (softmax max-subtraction is
      skipped: |logits| < ~3 for this model family), normalize with per-row
      reciprocal, DMA att out, PE-transpose att, ctx^T = vh.T-contraction.
      Heads are processed in pairs sharing the 128-partition dim (row/col
      tile packing).
  P3: out_partial = ctx^T.T @ Wo_slice.
"""

import sys

import numpy as np

if "/opt/trn_rl_repo" not in sys.path:
    sys.path.insert(0, "/opt/trn_rl_repo")

P = 128

# Full-problem config
FULL = dict(B=4, S=1024, E=1024, H=16, D=64, GROUPS=2)


def _cfg(B, S, E, H, D, GROUPS):
    DG = (H // GROUPS) * D  # d per core
    return dict(
        B=B, S=S, E=E, H=H, D=D, GROUPS=GROUPS,
        DG=DG,
        HL=H // GROUPS,       # local heads
        EC=E // P,            # E chunks (contraction tiles)
        ST=S // P,            # S tiles
        MT=DG // P,           # d m-tiles (2 heads per m-tile when D=64)
        NW=min(512, S),       # matmul moving width
    )


def build_core_kernel(use_f32r: bool = False, cfg: dict | None = None):
    """Build and compile the per-core Bass program (same program on all cores)."""
    import concourse.tile as tile
    from concourse import bacc, mybir
    from concourse.masks import make_identity

    c = _cfg(**(cfg or FULL))
    S, E, DG = c["S"], c["E"], c["DG"]
    EC, ST, MT, NW = c["EC"], c["ST"], c["MT"], c["NW"]
    HL = c["HL"]
    D = c["D"]
    assert D == 64

    f32 = mybir.dt.float32
    mm_dt = mybir.dt.float32r if use_f32r else f32
    Exp = mybir.ActivationFunctionType.Exp

    nc = bacc.Bacc("TRN2", target_bir_lowering=False, debug=False)

    q_ap = nc.dram_tensor("q", [S, E], f32, kind="ExternalInput").ap()
    k_ap = nc.dram_tensor("k", [S, E], f32, kind="ExternalInput").ap()
    v_ap = nc.dram_tensor("v", [S, E], f32, kind="ExternalInput").ap()
    wq_ap = nc.dram_tensor("wq", [E, DG], f32, kind="ExternalInput").ap()
    wk_ap = nc.dram_tensor("wk", [E, DG], f32, kind="ExternalInput").ap()
    wv_ap = nc.dram_tensor("wv", [E, DG], f32, kind="ExternalInput").ap()
    bq_ap = nc.dram_tensor("bq", [DG], f32, kind="ExternalInput").ap()
    bk_ap = nc.dram_tensor("bk", [DG], f32, kind="ExternalInput").ap()
    wo_ap = nc.dram_tensor("wo", [DG, E], f32, kind="ExternalInput").ap()
    att_ap = nc.dram_tensor("att", [HL, S, S], f32, kind="ExternalOutput").ap()
    outp_ap = nc.dram_tensor("outp", [S, E], f32, kind="ExternalOutput").ap()

    TQ = min(4, EC)   # transposes batched per PSUM tile
    SQ = min(4, ST)

    with tile.TileContext(nc) as tc:
        with (
            tc.tile_pool(name="const", bufs=1) as constp,
            tc.tile_pool(name="persist", bufs=1) as persist,
            tc.tile_pool(name="lp", bufs=8) as lp,
            tc.tile_pool(name="psA", bufs=2, space="PSUM") as psA,
            tc.tile_pool(name="psB", bufs=2, space="PSUM") as psB,
            tc.tile_pool(name="psC", bufs=2, space="PSUM") as psC,
        ):
            ident = constp.tile([P, P], f32)
            make_identity(nc, ident)

            bq_sb = constp.tile([P, MT], f32, tag="bq")
            bk_sb = constp.tile([P, MT], f32, tag="bk")
            nc.gpsimd.dma_start(bq_sb[:], bq_ap.rearrange("(m p) -> p m", p=P))
            nc.gpsimd.dma_start(bk_sb[:], bk_ap.rearrange("(m p) -> p m", p=P))

            qhT = persist.tile([P, MT, S], mm_dt, tag="qhT")
            khT = persist.tile([P, MT, S], mm_dt, tag="khT")
            vh = persist.tile([P, ST, DG], mm_dt, tag="vh")
            ctxT = persist.tile([P, MT, S], mm_dt, tag="ctxT")

            # ---------------- Phase 1: transpose inputs + projections ----------
            with (
                tc.tile_pool(name="xin", bufs=4) as xinp,
                tc.tile_pool(name="xt", bufs=2) as xtp,
                tc.tile_pool(name="w", bufs=2) as wp,
            ):
                for xi, (x_ap, w_ap2, out_sb, bias) in enumerate((
                    (q_ap, wq_ap, qhT, bq_sb),
                    (k_ap, wk_ap, khT, bk_sb),
                    (v_ap, wv_ap, vh, None),
                )):
                    _sid = nc.enter_named_scope(f"p1_{'qkv'[xi]}", False)[0]
                    w_sb = None

                    xT = xtp.tile([P, EC, S], mm_dt, tag="xT")
                    for st in range(ST):
                        xin = xinp.tile([P, E], f32, tag="xin")
                        # two half-DMAs: the first transpose quad only needs
                        # the first half, so PE starts ~0.7us sooner per chunk
                        nc.sync.dma_start(
                            xin[:, : E // 2], x_ap[st * P : (st + 1) * P, : E // 2]
                        )
                        nc.sync.dma_start(
                            xin[:, E // 2 :], x_ap[st * P : (st + 1) * P, E // 2 :]
                        )
                        if st == 0:
                            # issue the weight DMA behind the first input chunk
                            # so it doesn't head-block the transpose pipeline
                            w_sb = wp.tile([P, EC, DG], f32, tag="w")
                            nc.sync.dma_start(
                                w_sb[:], w_ap2.rearrange("(e p) d -> p e d", p=P)
                            )
                            if use_f32r:
                                w_mm = wp.tile([P, EC, DG], mm_dt, tag="wr")
                                nc.vector.tensor_copy(w_mm[:], w_sb[:])
                            else:
                                w_mm = w_sb
                        for eq in range(EC // TQ):
                            pst = psB.tile([P, TQ * P], f32, tag="psB")
                            for t in range(TQ):
                                e = eq * TQ + t
                                nc.tensor.transpose(
                                    pst[:, t * P : (t + 1) * P],
                                    xin[:, e * P : (e + 1) * P],
                                    ident[:],
                                )
                            nc.any.tensor_copy(
                                xT[:, eq * TQ : (eq + 1) * TQ, st * P : (st + 1) * P],
                                pst[:].rearrange("p (t c) -> p t c", c=P),
                            )

                    if bias is not None:
                        # transposed projection: out [d, s] = w.T @ x.T
                        for m in range(MT):
                            psp = psA.tile([P, S], f32, tag="psA")
                            for sh in range(S // NW):
                                for e in range(EC):
                                    nc.tensor.matmul(
                                        psp[:, sh * NW : (sh + 1) * NW],
                                        w_mm[:, e, m * P : (m + 1) * P],
                                        xT[:, e, sh * NW : (sh + 1) * NW],
                                        start=(e == 0),
                                        stop=(e == EC - 1),
                                    )
                            nc.vector.tensor_scalar_add(
                                out_sb[:, m, :], psp[:], bias[:, m : m + 1]
                            )
                    else:
                        # natural projection: out [s, d] = x @ w
                        for st in range(ST):
                            psp = psA.tile([P, S], f32, tag="psA")
                            for dh in range((DG + NW - 1) // NW):
                                nw = min(NW, DG - dh * NW)
                                for e in range(EC):
                                    nc.tensor.matmul(
                                        psp[:, dh * NW : dh * NW + nw],
                                        xT[:, e, st * P : (st + 1) * P],
                                        w_mm[:, e, dh * NW : dh * NW + nw],
                                        start=(e == 0),
                                        stop=(e == EC - 1),
                                    )
                            nc.vector.tensor_copy(out_sb[:, st, :], psp[:, :DG])
                    nc.leave_named_scope(f"p1_{'qkv'[xi]}", _sid, False)

            # ---------------- Phase 2: attention, head pairs -------------------
            with (
                tc.tile_pool(name="att", bufs=4) as attp,
                tc.tile_pool(name="attT", bufs=3) as attTp,
            ):
                for t in range(MT):
                    _sid2 = nc.enter_named_scope(f"p2_pair{t}", False)[0]
                    attT = [
                        attTp.tile([P, ST, S], mm_dt, tag="attT", name=f"attT_{t}_{hp}")
                        for hp in range(2)
                    ]
                    for i in range(ST):
                        for hp in range(2):
                            h = 2 * t + hp
                            hb = hp * 64
                            psl = psA.tile([P, S], f32, tag="psA")
                            for jh in range(S // NW):
                                nc.tensor.matmul(
                                    psl[:, jh * NW : (jh + 1) * NW],
                                    qhT[hb : hb + 64, t, i * P : (i + 1) * P],
                                    khT[hb : hb + 64, t, jh * NW : (jh + 1) * NW],
                                    start=True,
                                    stop=True,
                                )
                            e_t = attp.tile([P, S], f32, tag="att")
                            l_t = lp.tile([P, 1], f32, tag="l")
                            nc.scalar.activation(
                                e_t[:], psl[:], Exp, scale=0.125, accum_out=l_t[:]
                            )
                            rl_t = lp.tile([P, 1], f32, tag="rl")
                            nc.vector.reciprocal(rl_t[:], l_t[:])
                            a_t = attp.tile([P, S], f32, tag="att")
                            nc.any.tensor_scalar_mul(a_t[:], e_t[:], rl_t[:])
                            nc.sync.dma_start(
                                att_ap[h, i * P : (i + 1) * P, :], a_t[:]
                            )
                            for jq in range(ST // SQ):
                                pst = psB.tile([P, SQ * P], f32, tag="psB")
                                for tt in range(SQ):
                                    jt = jq * SQ + tt
                                    nc.tensor.transpose(
                                        pst[:, tt * P : (tt + 1) * P],
                                        a_t[:, jt * P : (jt + 1) * P],
                                        ident[:],
                                    )
                                nc.any.tensor_copy(
                                    attT[hp][
                                        :, jq * SQ : (jq + 1) * SQ, i * P : (i + 1) * P
                                    ],
                                    pst[:].rearrange("p (t c) -> p t c", c=P),
                                )
                    # ctx^T for the head pair, col-packed into one PSUM tile
                    for ih in range(S // NW):
                        psc = psC.tile([P, NW], f32, tag="psC")
                        for jt in range(ST):
                            for hp in range(2):
                                h = 2 * t + hp
                                hb = hp * 64
                                nc.tensor.matmul(
                                    psc[hb : hb + 64, :],
                                    vh[:, jt, h * 64 : (h + 1) * 64],
                                    attT[hp][:, jt, ih * NW : (ih + 1) * NW],
                                    start=(jt == 0),
                                    stop=(jt == ST - 1),
                                    tile_position=(0, hb) if hb else None,
                                    skip_group_check=True,
                                )
                        nc.vector.tensor_copy(
                            ctxT[:, t, ih * NW : (ih + 1) * NW], psc[:]
                        )
                    nc.leave_named_scope(f"p2_pair{t}", _sid2, False)

            # ---------------- Phase 3: output projection -----------------------
            with (
                tc.tile_pool(name="w3", bufs=1) as w3p,
                tc.tile_pool(name="outp", bufs=3) as outpp,
            ):
                _sid3 = nc.enter_named_scope("p3_outproj", False)[0]
                wo_sb = w3p.tile([P, MT, E], f32, tag="wo")
                nc.sync.dma_start(wo_sb[:], wo_ap.rearrange("(m p) o -> p m o", p=P))
                if use_f32r:
                    wo_mm = w3p.tile([P, MT, E], mm_dt, tag="wor")
                    nc.vector.tensor_copy(wo_mm[:], wo_sb[:])
                else:
                    wo_mm = wo_sb
                for i in range(ST):
                    pso = psA.tile([P, E], f32, tag="psA")
                    for oh in range(E // NW):
                        for m in range(MT):
                            nc.tensor.matmul(
                                pso[:, oh * NW : (oh + 1) * NW],
                                ctxT[:, m, i * P : (i + 1) * P],
                                wo_mm[:, m, oh * NW : (oh + 1) * NW],
                                start=(m == 0),
                                stop=(m == MT - 1),
                            )
                    o_t = outpp.tile([P, E], f32, tag="out")
                    nc.vector.tensor_copy(o_t[:], pso[:])
                    nc.sync.dma_start(outp_ap[i * P : (i + 1) * P, :], o_t[:])
                nc.leave_named_scope("p3_outproj", _sid3, False)

    nc.compile()
    return nc


_NC_CACHE: dict = {}


def _get_nc(use_f32r: bool = False):
    key = ("full", use_f32r)
    if key not in _NC_CACHE:
        _NC_CACHE[key] = build_core_kernel(use_f32r=use_f32r)
    return _NC_CACHE[key]


def kernel(q, k, v, Wq, bq, Wk, bk, Wv, bv, Wo, bo):
    from concourse.bass_utils import run_bass_kernel_spmd

    B, S, E = FULL["B"], FULL["S"], FULL["E"]
    H, GROUPS = FULL["H"], FULL["GROUPS"]
    HL = H // GROUPS
    DG = HL * FULL["D"]

    q = np.asarray(q, dtype=np.float32)
    k = np.asarray(k, dtype=np.float32)
    v = np.asarray(v, dtype=np.float32)
    Wq = np.asarray(Wq, dtype=np.float32)
    Wk = np.asarray(Wk, dtype=np.float32)
    Wv = np.asarray(Wv, dtype=np.float32)
    Wo = np.asarray(Wo, dtype=np.float32)
    bq = np.asarray(bq, dtype=np.float32)
    bk = np.asarray(bk, dtype=np.float32)
    bv = np.asarray(bv, dtype=np.float32)
    bo = np.asarray(bo, dtype=np.float32)

    nc = _get_nc()

    in_maps = []
    for c in range(8):
        b, g = divmod(c, 2)
        sl = slice(g * DG, (g + 1) * DG)
        in_maps.append(
            {
                "q": np.ascontiguousarray(q[b]),
                "k": np.ascontiguousarray(k[b]),
                "v": np.ascontiguousarray(v[b]),
                "wq": np.ascontiguousarray(Wq[:, sl]),
                "wk": np.ascontiguousarray(Wk[:, sl]),
                "wv": np.ascontiguousarray(Wv[:, sl]),
                "bq": np.ascontiguousarray(bq[sl]),
                "bk": np.ascontiguousarray(bk[sl]),
                "wo": np.ascontiguousarray(Wo[sl, :]),
            }
        )

    res = run_bass_kernel_spmd(nc, in_maps, list(range(8)))

    att = np.empty((B, H, S, S), dtype=np.float32)
    out = np.empty((B, S, E), dtype=np.float32)
    corr = (bv.astype(np.float64) @ Wo.astype(np.float64) + bo).astype(np.float32)
    for c in range(8):
        b, g = divmod(c, 2)
        att[b, g * HL : (g + 1) * HL] = res.results[c]["att"]
    for b in range(B):
        out[b] = res.results[2 * b]["outp"] + res.results[2 * b + 1]["outp"] + corr
    return out, att
